# revision 6
# baseline (speedup 1.0000x reference)
"""Boundary BCE loss kernel for Trainium2 (8 NeuronCores, data-parallel).

Computes mean(BCEWithLogits(pred, boundary(gt_mask))) where
boundary(m) = 1 iff the 3x3 neighborhood of a pixel (SAME window, valid
elements only) contains both a 0 and a 1.

Layout / algorithm (per core: 8 images of 1024x1024):
  - With *replicate* padding the value-set of a 3x3 window equals the set of
    valid in-bounds values. Weight the conv with center tap -8 (i.e.
    s' = replicate-pad 3x3 sum - 9*center): all-zeros -> s'=0, all-ones ->
    s'=9-9=0, mixed -> s' in [-8..-1] u [1..8]. So boundary z = (s' != 0),
    ONE vector op per tile, and sum(loss) = sum(softplus(x)) - sum(x*z).
  - The -9*center correction folds into the no-column-shift matmul's weight
    matrix (atop2/aint2/abst2), so the conv costs the same matmuls as the
    plain 3x3 count: 3 column-shifted matmuls per 512-col PSUM group (+ tiny
    N=1 edge-replicate matmuls reading mf's own edge columns).
  - Each image is row-tiled into 8 conv blocks of 128 input rows starting at
    126k (2-row overlap); block k=0 ("top") emits out rows 0..126 via a
    banded [128,127] matrix atop (replicate row -1 folded in), blocks
    k>=1 ("int") emit out rows 126k+1..126k+126 via aint whose column 126 is
    ZERO -- the resulting guaranteed s'=0 on partition 126 makes the x*z
    reduction contribute exactly 0 there, so reduction instructions can run
    on rectangular [127, 2048] tiles spanning a fused PAIR of blocks.
  - FUSION: blocks are processed in pairs (2pi, 2pi+1). One 3D-AP SWDGE
    *casting* DMA (int32->f8) loads both gt windows (row stride 126), so
    no engine ever spends time casting the mask; the two pred windows load
    as two plain 2D HWDGE DMAs (a 3D-AP HWDGE DMA costs ~4.2us of SP
    descriptor generation vs ~0.6us per 2D transfer).
  - exp/ln (softplus, Ln's free bias adds the +1) run fused [127,2048] for
    pairs pi>=1: partition 126 of each half then double-counts one row that
    the next block covers again; those sums land isolated in
    acc[126, fused-col] and the HOST subtracts that cell. Pair 0 runs
    exp/ln per-block (exact ranges), since its top half has no spare
    partition.
  - The 8 images' ragged bottom strips (16 in rows / 15 out rows each) are
    stacked into one [128, 1024] block via a 3D DMA and a block-diagonal
    matrix abst, exactly as a normal block.
  - TAIL: image 7's last two pairs are emitted as FOUR single-block units
    ([128,1024] tiles, exact exp/ln ranges), preceded by the bst unit. The
    last five units are all small, so the ACT engine keeps pace with the
    (end-of-stream bunched) x arrivals and the post-stream drain is one
    small unit (~3us) instead of ~7us.
  - Exp/Ln share one ACT table set (natural_log_exp_and_others; see
    _patch_act_tables) so tables load once. All six conv matrices arrive in
    one packed [128,768] DMA; all accumulators live in one [128,102] tile
    written back by a single output DMA.

Each core returns partials [128, 102]; the host sums in float64, subtracts
the fused-pair duplicate cells, and divides by N.
"""

import os
import sys
from collections import deque
from contextlib import ExitStack

import numpy as np

if "/opt/trn_rl_repo" not in sys.path and os.path.isdir("/opt/trn_rl_repo"):
    sys.path.append("/opt/trn_rl_repo")

N_CORES = 8
B, C, H, W = 64, 1, 1024, 1024
IMGS_PER_CORE = B // N_CORES  # 8
P = 128

N_PAIRS = 4          # fused block-pairs per image
MB = 15              # bottom strip out rows per image (1024 - (127+7*126))

# unit schedule: all pairs except img7-pi2/pi3; then the stacked bottom
# strips; then img7-pi2/pi3 as four single-block units. The last five units
# are all "small" (0.5MB x + 0.5MB mf, ~2.3us ACT work each): their x tiles
# arrive ~2.7us apart even when the mf queue drains early, so the ACT engine
# never accumulates backlog and the post-stream tail is one small unit long.
UNITS = [("pair", img, pi) for img in range(IMGS_PER_CORE)
         for pi in range(N_PAIRS) if not (img == IMGS_PER_CORE - 1 and
                                          pi >= N_PAIRS - 2)]
UNITS.append(("bst",))
UNITS.extend(("blk", k) for k in range(4))
N_UNITS = len(UNITS)                    # 35
SP_COLS = 2 * N_UNITS                   # softplus accum columns (2 per unit)
# units whose exp/ln run fused => host subtracts acc[126, 2*u]
FUSED_UNITS = [u for u, spec in enumerate(UNITS)
               if spec[0] == "pair" and spec[2] != 0]


def make_consts():
    """Banded vertical-conv matrices A[k, m] = weight of input row k in out m.
    The *2 variants subtract 9 at the center tap (in-row of out-row m) and are
    used for the no-column-shift matmul, yielding s' = 3x3sum - 9*center."""
    import ml_dtypes

    f8 = ml_dtypes.float8_e4m3fn

    atop = np.zeros((128, 127), np.float32)
    for m in range(127):
        for k in (m - 1, m, m + 1):
            if 0 <= k < 128:
                atop[k, m] += 1.0
    atop[0, 0] += 1.0  # replicate row -1 -> row 0
    atop2 = atop.copy()
    for m in range(127):
        atop2[m, m] -= 9.0

    aint = np.zeros((128, 127), np.float32)  # col 126 stays ZERO (guard)
    for m in range(126):
        for k in (m, m + 1, m + 2):
            aint[k, m] += 1.0
    aint2 = aint.copy()
    for m in range(126):
        aint2[m + 1, m] -= 9.0

    abot = np.zeros((MB + 1, MB), np.float32)
    for m in range(MB):
        for k in (m, m + 1, m + 2):
            if k <= MB:
                abot[k, m] += 1.0
    abot[MB, MB - 1] += 1.0  # replicate row h -> row h-1
    abot2 = abot.copy()
    for m in range(MB):
        abot2[m + 1, m] -= 9.0

    kbs = IMGS_PER_CORE * (MB + 1)
    mbs = IMGS_PER_CORE * MB
    abst = np.zeros((kbs, mbs), np.float32)
    abst2 = np.zeros((kbs, mbs), np.float32)
    for j in range(IMGS_PER_CORE):
        abst[j * (MB + 1) : (j + 1) * (MB + 1), j * MB : (j + 1) * MB] = abot
        abst2[j * (MB + 1) : (j + 1) * (MB + 1), j * MB : (j + 1) * MB] = abot2

    # pack all six into one [128, 768] tensor (128-col aligned views)
    conv_all = np.zeros((128, 768), np.float32)
    conv_all[:, 0:127] = atop
    conv_all[:, 128:255] = atop2
    conv_all[:, 256:383] = aint
    conv_all[:, 384:511] = aint2
    conv_all[:kbs, 512 : 512 + mbs] = abst
    conv_all[:kbs, 640 : 640 + mbs] = abst2

    return {"conv_all": conv_all.astype(f8)}


def build_program(nc, n_imgs=IMGS_PER_CORE, h=H, w=W):
    """Emit the per-core Tile program onto `nc` (a Bacc)."""
    import concourse.tile as tile
    from concourse import mybir
    from concourse.ap import AP

    f32 = mybir.dt.float32
    i32 = mybir.dt.int32
    bf16 = mybir.dt.bfloat16
    # never-read reduction outputs store as fp8: halves their SBUF write
    # bytes (which contend with DMA writes); the f32 accumulators carry the
    # real results, so these values are dead
    f8 = mybir.dt.float8e4

    rows = n_imgs * h
    kbs = n_imgs * (MB + 1)   # 128 stacked bottom-strip input rows
    mbs = n_imgs * MB         # 120 stacked bottom-strip output rows

    pred_d = nc.dram_tensor("pred", [rows, w], f32, kind="ExternalInput")
    gt_d = nc.dram_tensor("gt", [rows, w], i32, kind="ExternalInput")
    call_d = nc.dram_tensor("conv_all", [128, 768], f8, kind="ExternalInput")
    # partials: [0:SP_COLS) softplus sums, then N_UNITS x*z sums
    out_d = nc.dram_tensor("partials", [P, SP_COLS + N_UNITS], f32,
                           kind="ExternalOutput")

    pred = pred_d.ap()
    gt = gt_d.ap()
    pred3 = pred.rearrange("(j r) c -> j r c", j=n_imgs)
    gt3 = gt.rearrange("(j r) c -> j r c", j=n_imgs)
    out = out_d.ap()

    Exp = mybir.ActivationFunctionType.Exp
    Ln = mybir.ActivationFunctionType.Ln
    NE = mybir.AluOpType.not_equal
    MUL = mybir.AluOpType.mult

    with tile.TileContext(nc) as tc, ExitStack() as ctx:
        consts = ctx.enter_context(tc.tile_pool(name="consts", bufs=1))
        xs = ctx.enter_context(tc.tile_pool(name="xs", bufs=12))
        mfs = ctx.enter_context(tc.tile_pool(name="mfs", bufs=6))
        exs = ctx.enter_context(tc.tile_pool(name="exs", bufs=2))
        sps = ctx.enter_context(tc.tile_pool(name="sps", bufs=2))
        ws = ctx.enter_context(tc.tile_pool(name="ws", bufs=4))
        accp = ctx.enter_context(tc.tile_pool(name="accs", bufs=1))
        psum = ctx.enter_context(tc.tile_pool(name="psum", bufs=2, space="PSUM"))

        # one packed DMA on the scalar HWDGE ring: descriptor-gen must not
        # delay unit 0's x loads on the sync ring (ACT is idle until the
        # first x tile lands anyway)
        ct = consts.tile([128, 768], f8, tag="conv_all")
        nc.scalar.dma_start(ct[:], call_d.ap()[:])
        atop, atop2 = ct[:, 0:127], ct[:, 128:255]
        aint, aint2 = ct[:, 256:383], ct[:, 384:511]
        abst, abst2 = ct[:, 512 : 512 + mbs], ct[:, 640 : 640 + mbs]

        # single accumulator tile: softplus cols then x*z cols -> 1 out DMA
        acc = accp.tile([P, SP_COLS + N_UNITS], f32, tag="acc")
        nc.vector.memset(acc[:], 0.0)

        def conv_half(s2, ac, ao, mf2, Bo):
            """3x3 weighted conv of one 1024-col half: per 512-col PSUM group
            a center matmul with ac (= a - 9*centerband) + 2 column-shifted
            matmuls with ao + N=1 edge-replicate matmuls from mf's own edges.
            (Matmuls wider than 512 or not bank-aligned fail the ISA check
            s3d3_mm_num_elements, so per-512-col emission is mandatory.)"""
            mm = nc.tensor.matmul
            mm(s2[:, Bo + 0 : Bo + 512], ac[:], mf2[:, Bo + 0 : Bo + 512],
               start=True, stop=False)
            mm(s2[:, Bo + 0 : Bo + 512], ao[:], mf2[:, Bo + 1 : Bo + 513],
               start=False, stop=False)
            mm(s2[:, Bo + 1 : Bo + 512], ao[:], mf2[:, Bo + 0 : Bo + 511],
               start=False, stop=False)
            mm(s2[:, Bo + 0 : Bo + 1], ao[:], mf2[:, Bo + 0 : Bo + 1],
               start=False, stop=True)
            mm(s2[:, Bo + 512 : Bo + 1024], ac[:], mf2[:, Bo + 512 : Bo + 1024],
               start=True, stop=False)
            mm(s2[:, Bo + 512 : Bo + 1024], ao[:], mf2[:, Bo + 511 : Bo + 1023],
               start=False, stop=False)
            mm(s2[:, Bo + 512 : Bo + 1023], ao[:], mf2[:, Bo + 513 : Bo + 1024],
               start=False, stop=False)
            mm(s2[:, Bo + 1023 : Bo + 1024], ao[:], mf2[:, Bo + 1023 : Bo + 1024],
               start=False, stop=True)

        def front_pair(img, pi):
            """Loads + conv for fused block pair (2pi, 2pi+1) of one image."""
            in_r0 = img * h + 252 * pi
            mf2 = mfs.tile([128, 2048], f8, tag="mf")
            nc.gpsimd.dma_start(
                mf2[:],
                AP(gt.tensor, in_r0 * w, [(w, 128), (126 * w, 2), (1, w)]),
            )
            or0 = 0 if pi == 0 else 126 * 2 * pi + 1
            dlt = 127 if pi == 0 else 126
            xr0 = img * h + or0
            # two plain 2D loads: a single 3D-AP HWDGE DMA costs ~4.2us of SP
            # descriptor generation (vs ~0.7us per 2D), throttling the x feed.
            # 127 rows each: partition 127 is never read by any consumer.
            x2 = xs.tile([128, 2048], f32, tag="x")
            nc.sync.dma_start(x2[0:127, 0:1024], pred[xr0 : xr0 + 127, :])
            nc.sync.dma_start(x2[0:127, 1024:2048],
                              pred[xr0 + dlt : xr0 + dlt + 127, :])
            s2 = psum.tile([127, 2048], f32, tag="s")
            conv_half(s2, atop2 if pi == 0 else aint2,
                      atop if pi == 0 else aint, mf2, 0)
            conv_half(s2, aint2, aint, mf2, 1024)
            return s2, x2

        def front_bst():
            """Loads + conv for the stacked bottom strips of all 8 images."""
            mfb = mfs.tile([kbs, w], f8, tag="mf")
            nc.gpsimd.dma_start(mfb[:], gt3[:, h - (MB + 1) : h, :])
            xb = xs.tile([mbs, w], f32, tag="x")
            nc.sync.dma_start(xb[:], pred3[:, h - MB : h, :])
            sb = psum.tile([mbs, w], f32, tag="s")
            conv_half(sb, abst2, abst, mfb, 0)
            return sb, xb

        def front_blk(k):
            """Loads + conv for single block 4+k of the last image (tail)."""
            img = n_imgs - 1
            in_r0 = img * h + 252 * (N_PAIRS - 2) + 126 * k
            mf2 = mfs.tile([128, 2048], f8, tag="mf")
            nc.gpsimd.dma_start(
                mf2[:, 0:1024], AP(gt.tensor, in_r0 * w, [(w, 128), (1, w)])
            )
            xr0 = in_r0 + 1
            x2 = xs.tile([128, 2048], f32, tag="x")
            nc.sync.dma_start(x2[0:127, 0:1024], pred[xr0 : xr0 + 127, :])
            s2 = psum.tile([127, 2048], f32, tag="s")
            conv_half(s2, aint2, aint, mf2, 0)
            return s2, x2

        def red_xz(u, s2, x2, np_, nc_):
            """acc[:, SP_COLS+u] += sum_cols x * (s' != 0) over [np_, nc_]."""
            w1 = ws.tile([127, 2048], f8, tag="w1")
            nc.vector.scalar_tensor_tensor(
                w1[0:np_, 0:nc_], s2[0:np_, 0:nc_], 0.0, x2[0:np_, 0:nc_],
                NE, MUL,
                accum_out=acc[0:np_, SP_COLS + u : SP_COLS + u + 1],
            )

        def back_pair(u, fused, s2, x2):
            """Reductions for one fused pair: softplus sums + x*z sums."""
            # exp output in bf16: halves ACT's SBUF write+read bytes (which
            # contend with DMA writes in slow-HBM phases); e^x rounding is
            # random-sign and vanishes in the 67M-pixel mean
            ex2 = exs.tile([127, 2048], bf16, tag="ex")
            sp2 = sps.tile([127, 2048], f8, tag="sp")
            if fused:
                # partition 126 double-counts one row per half; the host
                # subtracts acc[126, 2u] (it contains ONLY those rows)
                nc.scalar.activation(ex2[:], x2[0:127, :], Exp)
                nc.scalar.activation(sp2[:], ex2[:], Ln, bias=1.0,
                                     accum_out=acc[0:127, 2 * u : 2 * u + 1])
            else:
                nc.scalar.activation(ex2[0:127, 0:1024], x2[0:127, 0:1024], Exp)
                nc.scalar.activation(ex2[0:126, 1024:2048], x2[0:126, 1024:2048],
                                     Exp)
                nc.scalar.activation(sp2[0:127, 0:1024], ex2[0:127, 0:1024], Ln,
                                     bias=1.0,
                                     accum_out=acc[0:127, 2 * u : 2 * u + 1])
                nc.scalar.activation(sp2[0:126, 1024:2048], ex2[0:126, 1024:2048],
                                     Ln, bias=1.0,
                                     accum_out=acc[0:126, 2 * u + 1 : 2 * u + 2])
            red_xz(u, s2, x2, 127, 2048)

        def back_bst(u, sb, xb):
            ex = exs.tile([127, 2048], bf16, tag="ex")
            sp = sps.tile([127, 2048], f8, tag="sp")
            nc.scalar.activation(ex[0:mbs, 0:1024], xb[:], Exp)
            nc.scalar.activation(sp[0:mbs, 0:1024], ex[0:mbs, 0:1024], Ln,
                                 bias=1.0,
                                 accum_out=acc[0:mbs, 2 * u : 2 * u + 1])
            w1 = ws.tile([127, 2048], f8, tag="w1")
            nc.vector.scalar_tensor_tensor(
                w1[0:mbs, 0:1024], sb[:], 0.0, xb[:], NE, MUL,
                accum_out=acc[0:mbs, SP_COLS + u : SP_COLS + u + 1],
            )

        def back_blk(u, s2, x2):
            """Single tail block: exact exp/ln over its 126 out rows."""
            ex = exs.tile([127, 2048], bf16, tag="ex")
            sp = sps.tile([127, 2048], f8, tag="sp")
            nc.scalar.activation(ex[0:126, 0:1024], x2[0:126, 0:1024], Exp)
            nc.scalar.activation(sp[0:126, 0:1024], ex[0:126, 0:1024], Ln,
                                 bias=1.0,
                                 accum_out=acc[0:126, 2 * u : 2 * u + 1])
            red_xz(u, s2, x2, 127, 1024)

        pending = deque()
        for u, spec in enumerate(UNITS):
            if spec[0] == "pair":
                _, img, pi = spec
                pending.append(("pair", u, pi != 0, front_pair(img, pi)))
            elif spec[0] == "bst":
                pending.append(("bst", u, False, front_bst()))
            else:
                pending.append(("blk", u, False, front_blk(spec[1])))
            if len(pending) > 2:
                kind, pu, fused, pf = pending.popleft()
                if kind == "pair":
                    back_pair(pu, fused, *pf)
                elif kind == "bst":
                    back_bst(pu, *pf)
                else:
                    back_blk(pu, *pf)
        while pending:
            kind, pu, fused, pf = pending.popleft()
            if kind == "pair":
                back_pair(pu, fused, *pf)
            elif kind == "bst":
                back_bst(pu, *pf)
            else:
                back_blk(pu, *pf)

        nc.sync.dma_start(out[:], acc[:])


def _patch_act_tables():
    """Make Exp and Ln resolve to the one table set containing both
    (natural_log_exp_and_others); otherwise the table-load pass alternates
    between exp_and_others and natural_log, reloading ~1.3us per activation.
    Set indices (= positions in act_info.json's act_func_sets) are preserved;
    only the membership used for set *selection* is filtered."""
    import concourse.bacc as bacc_mod
    from concourse import mybir

    if getattr(bacc_mod, "_act_tables_patched", False):
        return
    orig = bacc_mod.get_activation_tables
    exp_ln = {mybir.ActivationFunctionType.Exp, mybir.ActivationFunctionType.Ln}

    def patched(arch):
        out = {}
        for name, fns in orig(arch).items():
            out[name] = set(fns) if name == "natural_log_exp_and_others" else (
                set(fns) - exp_ln
            )
        return out

    bacc_mod.get_activation_tables = patched
    bacc_mod._act_tables_patched = True


def _ensure_ntff_hook():
    """Best-effort: make run_bass_kernel_spmd(trace=True) usable. The agent
    container ships no antenv.axon_hooks module, so a BASS_TRACE=1 run would
    otherwise die on the import inside bass_utils. Harmless if unused."""
    try:
        import types

        import antenv

        if "antenv.axon_hooks" in sys.modules:
            return
        m = types.ModuleType("antenv.axon_hooks")
        _h = {}
        m.set_axon_ntff_profile_hook = lambda h: _h.__setitem__("h", h)
        m.get_axon_ntff_profile_hook = lambda: _h.get("h")
        sys.modules["antenv.axon_hooks"] = m
        antenv.axon_hooks = m
        try:
            from trn_agent_boot.trn_boot import _ntff_profile_via_ctypes

            so = "/opt/axon/libaxon_pjrt.so"
            if os.path.exists(so):
                m.set_axon_ntff_profile_hook(_ntff_profile_via_ctypes(so))
        except Exception:
            pass
        try:
            import concourse.bass_utils as bu

            bu.upload_artifacts = lambda tmpdir: tmpdir
        except Exception:
            pass
    except Exception:
        pass


_CACHE = {}


def _get_nc():
    if "nc" not in _CACHE:
        import concourse.bacc as bacc

        _ensure_ntff_hook()
        _patch_act_tables()
        nc = bacc.Bacc("TRN2", target_bir_lowering=False, debug=False,
                       num_devices=N_CORES)
        build_program(nc)
        nc.compile()
        _CACHE["nc"] = nc
    return _CACHE["nc"]


def kernel(pred_boundary: np.ndarray, gt_mask: np.ndarray) -> np.ndarray:
    from concourse.bass_utils import run_bass_kernel_spmd

    nc = _get_nc()
    consts = make_consts()

    pred = np.ascontiguousarray(pred_boundary, dtype=np.float32).reshape(B * H, W)
    gt = np.ascontiguousarray(gt_mask, dtype=np.int32).reshape(B * H, W)

    rows_per_core = IMGS_PER_CORE * H
    in_maps = []
    for c in range(N_CORES):
        r0 = c * rows_per_core
        in_maps.append(
            {
                "pred": pred[r0 : r0 + rows_per_core],
                "gt": gt[r0 : r0 + rows_per_core],
                **consts,
            }
        )

    res = run_bass_kernel_spmd(nc, in_maps, list(range(N_CORES)))
    _CACHE["last_results"] = res

    fused_sp_cols = [2 * u for u in FUSED_UNITS]
    total = np.float64(0.0)
    for c in range(N_CORES):
        p = res.results[c]["partials"].astype(np.float64)
        sp = p[:, 0:SP_COLS].sum() - p[126, fused_sp_cols].sum()
        xz = p[:, SP_COLS : SP_COLS + N_UNITS].sum()
        total += sp - xz

    mean = total / float(B * C * H * W)
    return np.float32(mean)


# revision 8
# speedup vs baseline: 6.9278x; 6.9278x over previous
"""Boundary BCE loss kernel for Trainium2 (8 NeuronCores, data-parallel).

Computes mean(BCEWithLogits(pred, boundary(gt_mask))) where
boundary(m) = 1 iff the 3x3 neighborhood of a pixel (SAME window, valid
elements only) contains both a 0 and a 1.

Layout / algorithm (per core: 8 images of 1024x1024):
  - With *replicate* padding the value-set of a 3x3 window equals the set of
    valid in-bounds values. Weight the conv with center tap -8 (i.e.
    s' = replicate-pad 3x3 sum - 9*center): all-zeros -> s'=0, all-ones ->
    s'=9-9=0, mixed -> s' in [-8..-1] u [1..8]. So boundary z = (s' != 0),
    ONE vector op per tile, and sum(loss) = sum(softplus(x)) - sum(x*z).
  - The -9*center correction folds into the no-column-shift matmul's weight
    matrix (atop2/aint2/abst2), so the conv costs the same matmuls as the
    plain 3x3 count: 3 column-shifted matmuls per 512-col PSUM group (+ tiny
    N=1 edge-replicate matmuls reading mf's own edge columns).
  - Each image is row-tiled into 8 conv blocks of 128 input rows starting at
    126k (2-row overlap); block k=0 ("top") emits out rows 0..126 via a
    banded [128,127] matrix atop (replicate row -1 folded in), blocks
    k>=1 ("int") emit out rows 126k+1..126k+126 via aint whose column 126 is
    ZERO -- the resulting guaranteed s'=0 on partition 126 makes the x*z
    reduction contribute exactly 0 there, so reduction instructions can run
    on rectangular [127, 2048] tiles spanning a fused PAIR of blocks.
  - FUSION: blocks are processed in pairs (2pi, 2pi+1). One 3D-AP SWDGE
    *casting* DMA (int32->f8) loads both gt windows (row stride 126), so
    no engine ever spends time casting the mask; the two pred windows load
    as two plain 2D HWDGE DMAs (a 3D-AP HWDGE DMA costs ~4.2us of SP
    descriptor generation vs ~0.6us per 2D transfer).
  - exp/ln (softplus, Ln's free bias adds the +1) run fused [127,2048] for
    pairs pi>=1: partition 126 of each half then double-counts one row that
    the next block covers again; those sums land isolated in
    acc[126, fused-col] and the HOST subtracts that cell. Pair 0 runs
    exp/ln per-block (exact ranges), since its top half has no spare
    partition.
  - The 8 images' ragged bottom strips (16 in rows / 15 out rows each) are
    stacked into one [128, 1024] block via a 3D DMA and a block-diagonal
    matrix abst, exactly as a normal block.
  - TAIL: image 7's last two pairs are emitted as FOUR single-block units
    ([128,1024] tiles, exact exp/ln ranges), preceded by the bst unit. The
    last five units are all small, so the ACT engine keeps pace with the
    (end-of-stream bunched) x arrivals and the post-stream drain is one
    small unit (~3us) instead of ~7us.
  - Exp/Ln share one ACT table set (natural_log_exp_and_others; see
    _patch_act_tables) so tables load once. All six conv matrices arrive in
    one packed [128,768] DMA; all accumulators live in one [128,102] tile
    written back by a single output DMA.

Each core returns partials [128, 102]; the host sums in float64, subtracts
the fused-pair duplicate cells, and divides by N.
"""

import os
import sys
from collections import deque
from contextlib import ExitStack

import numpy as np

if "/opt/trn_rl_repo" not in sys.path and os.path.isdir("/opt/trn_rl_repo"):
    sys.path.append("/opt/trn_rl_repo")

N_CORES = 8
B, C, H, W = 64, 1, 1024, 1024
IMGS_PER_CORE = B // N_CORES  # 8
P = 128

N_PAIRS = 4          # fused block-pairs per image
MB = 15              # bottom strip out rows per image (1024 - (127+7*126))

# unit schedule: all pairs except img7-pi2/pi3; then the stacked bottom
# strips; then img7-pi2/pi3 as four single-block units. The last five units
# are all "small" (0.5MB x + 0.5MB mf, ~2.3us ACT work each): their x tiles
# arrive ~2.7us apart even when the mf queue drains early, so the ACT engine
# never accumulates backlog and the post-stream tail is one small unit long.
UNITS = [("pair", img, pi) for img in range(IMGS_PER_CORE)
         for pi in range(N_PAIRS) if not (img == IMGS_PER_CORE - 1 and
                                          pi >= N_PAIRS - 2)]
UNITS.append(("bst",))
UNITS.extend(("blk", k) for k in range(4))
N_UNITS = len(UNITS)                    # 35
SP_COLS = 2 * N_UNITS                   # softplus accum columns (2 per unit)
# units whose exp/ln run fused => host subtracts acc[126, 2*u]
FUSED_UNITS = [u for u, spec in enumerate(UNITS)
               if spec[0] == "pair" and spec[2] != 0]


def make_consts():
    """Banded vertical-conv matrices A[k, m] = weight of input row k in out m.
    The *2 variants subtract 9 at the center tap (in-row of out-row m) and are
    used for the no-column-shift matmul, yielding s' = 3x3sum - 9*center."""
    import ml_dtypes

    f8 = ml_dtypes.float8_e4m3fn

    atop = np.zeros((128, 127), np.float32)
    for m in range(127):
        for k in (m - 1, m, m + 1):
            if 0 <= k < 128:
                atop[k, m] += 1.0
    atop[0, 0] += 1.0  # replicate row -1 -> row 0
    atop2 = atop.copy()
    for m in range(127):
        atop2[m, m] -= 9.0

    aint = np.zeros((128, 127), np.float32)  # col 126 stays ZERO (guard)
    for m in range(126):
        for k in (m, m + 1, m + 2):
            aint[k, m] += 1.0
    aint2 = aint.copy()
    for m in range(126):
        aint2[m + 1, m] -= 9.0

    abot = np.zeros((MB + 1, MB), np.float32)
    for m in range(MB):
        for k in (m, m + 1, m + 2):
            if k <= MB:
                abot[k, m] += 1.0
    abot[MB, MB - 1] += 1.0  # replicate row h -> row h-1
    abot2 = abot.copy()
    for m in range(MB):
        abot2[m + 1, m] -= 9.0

    kbs = IMGS_PER_CORE * (MB + 1)
    mbs = IMGS_PER_CORE * MB
    abst = np.zeros((kbs, mbs), np.float32)
    abst2 = np.zeros((kbs, mbs), np.float32)
    for j in range(IMGS_PER_CORE):
        abst[j * (MB + 1) : (j + 1) * (MB + 1), j * MB : (j + 1) * MB] = abot
        abst2[j * (MB + 1) : (j + 1) * (MB + 1), j * MB : (j + 1) * MB] = abot2

    # pack all six into one [128, 768] tensor (128-col aligned views)
    conv_all = np.zeros((128, 768), np.float32)
    conv_all[:, 0:127] = atop
    conv_all[:, 128:255] = atop2
    conv_all[:, 256:383] = aint
    conv_all[:, 384:511] = aint2
    conv_all[:kbs, 512 : 512 + mbs] = abst
    conv_all[:kbs, 640 : 640 + mbs] = abst2

    return {"conv_all": conv_all.astype(f8)}


def build_program(nc, n_imgs=IMGS_PER_CORE, h=H, w=W):
    """Emit the per-core Tile program onto `nc` (a Bacc)."""
    import concourse.tile as tile
    from concourse import mybir
    from concourse.ap import AP

    f32 = mybir.dt.float32
    i32 = mybir.dt.int32
    bf16 = mybir.dt.bfloat16
    # never-read reduction outputs store as fp8: halves their SBUF write
    # bytes (which contend with DMA writes); the f32 accumulators carry the
    # real results, so these values are dead
    f8 = mybir.dt.float8e4

    rows = n_imgs * h
    kbs = n_imgs * (MB + 1)   # 128 stacked bottom-strip input rows
    mbs = n_imgs * MB         # 120 stacked bottom-strip output rows

    pred_d = nc.dram_tensor("pred", [rows, w], f32, kind="ExternalInput")
    gt_d = nc.dram_tensor("gt", [rows, w], i32, kind="ExternalInput")
    call_d = nc.dram_tensor("conv_all", [128, 768], f8, kind="ExternalInput")
    # partials: [0:SP_COLS) softplus sums, then N_UNITS x*z sums
    out_d = nc.dram_tensor("partials", [P, SP_COLS + N_UNITS], f32,
                           kind="ExternalOutput")

    pred = pred_d.ap()
    gt = gt_d.ap()
    pred3 = pred.rearrange("(j r) c -> j r c", j=n_imgs)
    gt3 = gt.rearrange("(j r) c -> j r c", j=n_imgs)
    out = out_d.ap()

    Exp = mybir.ActivationFunctionType.Exp
    Ln = mybir.ActivationFunctionType.Ln
    NE = mybir.AluOpType.not_equal
    MUL = mybir.AluOpType.mult

    with tile.TileContext(nc) as tc, ExitStack() as ctx:
        consts = ctx.enter_context(tc.tile_pool(name="consts", bufs=1))
        xs = ctx.enter_context(tc.tile_pool(name="xs", bufs=12))
        mfs = ctx.enter_context(tc.tile_pool(name="mfs", bufs=6))
        exs = ctx.enter_context(tc.tile_pool(name="exs", bufs=2))
        sps = ctx.enter_context(tc.tile_pool(name="sps", bufs=2))
        ws = ctx.enter_context(tc.tile_pool(name="ws", bufs=4))
        accp = ctx.enter_context(tc.tile_pool(name="accs", bufs=1))
        psum = ctx.enter_context(tc.tile_pool(name="psum", bufs=2, space="PSUM"))

        # one packed DMA on the scalar HWDGE ring: descriptor-gen must not
        # delay unit 0's x loads on the sync ring (ACT is idle until the
        # first x tile lands anyway)
        ct = consts.tile([128, 768], f8, tag="conv_all")
        nc.scalar.dma_start(ct[:], call_d.ap()[:])
        atop, atop2 = ct[:, 0:127], ct[:, 128:255]
        aint, aint2 = ct[:, 256:383], ct[:, 384:511]
        abst, abst2 = ct[:, 512 : 512 + mbs], ct[:, 640 : 640 + mbs]

        # single accumulator tile: softplus cols then x*z cols -> 1 out DMA
        acc = accp.tile([P, SP_COLS + N_UNITS], f32, tag="acc")
        nc.vector.memset(acc[:], 0.0)

        def conv_half(s2, ac, ao, mf2, Bo):
            """3x3 weighted conv of one 1024-col half: per 512-col PSUM group
            a center matmul with ac (= a - 9*centerband) + 2 column-shifted
            matmuls with ao + N=1 edge-replicate matmuls from mf's own edges.
            (Matmuls wider than 512 or not bank-aligned fail the ISA check
            s3d3_mm_num_elements, so per-512-col emission is mandatory.)"""
            mm = nc.tensor.matmul
            mm(s2[:, Bo + 0 : Bo + 512], ac[:], mf2[:, Bo + 0 : Bo + 512],
               start=True, stop=False)
            mm(s2[:, Bo + 0 : Bo + 512], ao[:], mf2[:, Bo + 1 : Bo + 513],
               start=False, stop=False)
            mm(s2[:, Bo + 1 : Bo + 512], ao[:], mf2[:, Bo + 0 : Bo + 511],
               start=False, stop=False)
            mm(s2[:, Bo + 0 : Bo + 1], ao[:], mf2[:, Bo + 0 : Bo + 1],
               start=False, stop=True)
            mm(s2[:, Bo + 512 : Bo + 1024], ac[:], mf2[:, Bo + 512 : Bo + 1024],
               start=True, stop=False)
            mm(s2[:, Bo + 512 : Bo + 1024], ao[:], mf2[:, Bo + 511 : Bo + 1023],
               start=False, stop=False)
            mm(s2[:, Bo + 512 : Bo + 1023], ao[:], mf2[:, Bo + 513 : Bo + 1024],
               start=False, stop=False)
            mm(s2[:, Bo + 1023 : Bo + 1024], ao[:], mf2[:, Bo + 1023 : Bo + 1024],
               start=False, stop=True)

        def front_pair(img, pi):
            """Loads + conv for fused block pair (2pi, 2pi+1) of one image."""
            in_r0 = img * h + 252 * pi
            mf2 = mfs.tile([128, 2048], f8, tag="mf")
            nc.gpsimd.dma_start(
                mf2[:],
                AP(gt.tensor, in_r0 * w, [(w, 128), (126 * w, 2), (1, w)]),
            )
            or0 = 0 if pi == 0 else 126 * 2 * pi + 1
            dlt = 127 if pi == 0 else 126
            xr0 = img * h + or0
            # two plain 2D loads: a single 3D-AP HWDGE DMA costs ~4.2us of SP
            # descriptor generation (vs ~0.7us per 2D), throttling the x feed.
            # NOTE: loads must be full 128 rows -- a 127-row transfer drops
            # off the DMA_DIRECT2D fast path and costs ~20us of SP time each.
            x2 = xs.tile([128, 2048], f32, tag="x")
            nc.sync.dma_start(x2[:, 0:1024], pred[xr0 : xr0 + 128, :])
            nc.sync.dma_start(x2[:, 1024:2048],
                              pred[xr0 + dlt : xr0 + dlt + 128, :])
            s2 = psum.tile([127, 2048], f32, tag="s")
            conv_half(s2, atop2 if pi == 0 else aint2,
                      atop if pi == 0 else aint, mf2, 0)
            conv_half(s2, aint2, aint, mf2, 1024)
            return s2, x2

        def front_bst():
            """Loads + conv for the stacked bottom strips of all 8 images."""
            mfb = mfs.tile([kbs, w], f8, tag="mf")
            nc.gpsimd.dma_start(mfb[:], gt3[:, h - (MB + 1) : h, :])
            xb = xs.tile([mbs, w], f32, tag="x")
            nc.sync.dma_start(xb[:], pred3[:, h - MB : h, :])
            sb = psum.tile([mbs, w], f32, tag="s")
            conv_half(sb, abst2, abst, mfb, 0)
            return sb, xb

        def front_blk(k):
            """Loads + conv for single block 4+k of the last image (tail)."""
            img = n_imgs - 1
            in_r0 = img * h + 252 * (N_PAIRS - 2) + 126 * k
            mf2 = mfs.tile([128, 2048], f8, tag="mf")
            nc.gpsimd.dma_start(
                mf2[:, 0:1024], AP(gt.tensor, in_r0 * w, [(w, 128), (1, w)])
            )
            xr0 = in_r0 + 1
            x2 = xs.tile([128, 2048], f32, tag="x")
            nc.sync.dma_start(x2[:, 0:1024], pred[xr0 : xr0 + 128, :])
            s2 = psum.tile([127, 2048], f32, tag="s")
            conv_half(s2, aint2, aint, mf2, 0)
            return s2, x2

        def red_xz(u, s2, x2, np_, nc_):
            """acc[:, SP_COLS+u] += sum_cols x * (s' != 0) over [np_, nc_]."""
            w1 = ws.tile([127, 2048], f8, tag="w1")
            nc.vector.scalar_tensor_tensor(
                w1[0:np_, 0:nc_], s2[0:np_, 0:nc_], 0.0, x2[0:np_, 0:nc_],
                NE, MUL,
                accum_out=acc[0:np_, SP_COLS + u : SP_COLS + u + 1],
            )

        def back_pair(u, fused, s2, x2):
            """Reductions for one fused pair: softplus sums + x*z sums."""
            # exp output in bf16: halves ACT's SBUF write+read bytes (which
            # contend with DMA writes in slow-HBM phases); e^x rounding is
            # random-sign and vanishes in the 67M-pixel mean
            ex2 = exs.tile([127, 2048], bf16, tag="ex")
            sp2 = sps.tile([127, 2048], f8, tag="sp")
            if fused:
                # partition 126 double-counts one row per half; the host
                # subtracts acc[126, 2u] (it contains ONLY those rows)
                nc.scalar.activation(ex2[:], x2[0:127, :], Exp)
                nc.scalar.activation(sp2[:], ex2[:], Ln, bias=1.0,
                                     accum_out=acc[0:127, 2 * u : 2 * u + 1])
            else:
                nc.scalar.activation(ex2[0:127, 0:1024], x2[0:127, 0:1024], Exp)
                nc.scalar.activation(ex2[0:126, 1024:2048], x2[0:126, 1024:2048],
                                     Exp)
                nc.scalar.activation(sp2[0:127, 0:1024], ex2[0:127, 0:1024], Ln,
                                     bias=1.0,
                                     accum_out=acc[0:127, 2 * u : 2 * u + 1])
                nc.scalar.activation(sp2[0:126, 1024:2048], ex2[0:126, 1024:2048],
                                     Ln, bias=1.0,
                                     accum_out=acc[0:126, 2 * u + 1 : 2 * u + 2])
            red_xz(u, s2, x2, 127, 2048)

        def back_bst(u, sb, xb):
            ex = exs.tile([127, 2048], bf16, tag="ex")
            sp = sps.tile([127, 2048], f8, tag="sp")
            nc.scalar.activation(ex[0:mbs, 0:1024], xb[:], Exp)
            nc.scalar.activation(sp[0:mbs, 0:1024], ex[0:mbs, 0:1024], Ln,
                                 bias=1.0,
                                 accum_out=acc[0:mbs, 2 * u : 2 * u + 1])
            w1 = ws.tile([127, 2048], f8, tag="w1")
            nc.vector.scalar_tensor_tensor(
                w1[0:mbs, 0:1024], sb[:], 0.0, xb[:], NE, MUL,
                accum_out=acc[0:mbs, SP_COLS + u : SP_COLS + u + 1],
            )

        def back_blk(u, s2, x2):
            """Single tail block: exact exp/ln over its 126 out rows."""
            ex = exs.tile([127, 2048], bf16, tag="ex")
            sp = sps.tile([127, 2048], f8, tag="sp")
            nc.scalar.activation(ex[0:126, 0:1024], x2[0:126, 0:1024], Exp)
            nc.scalar.activation(sp[0:126, 0:1024], ex[0:126, 0:1024], Ln,
                                 bias=1.0,
                                 accum_out=acc[0:126, 2 * u : 2 * u + 1])
            red_xz(u, s2, x2, 127, 1024)

        pending = deque()
        for u, spec in enumerate(UNITS):
            if spec[0] == "pair":
                _, img, pi = spec
                pending.append(("pair", u, pi != 0, front_pair(img, pi)))
            elif spec[0] == "bst":
                pending.append(("bst", u, False, front_bst()))
            else:
                pending.append(("blk", u, False, front_blk(spec[1])))
            if len(pending) > 2:
                kind, pu, fused, pf = pending.popleft()
                if kind == "pair":
                    back_pair(pu, fused, *pf)
                elif kind == "bst":
                    back_bst(pu, *pf)
                else:
                    back_blk(pu, *pf)
        while pending:
            kind, pu, fused, pf = pending.popleft()
            if kind == "pair":
                back_pair(pu, fused, *pf)
            elif kind == "bst":
                back_bst(pu, *pf)
            else:
                back_blk(pu, *pf)

        nc.sync.dma_start(out[:], acc[:])


def _patch_act_tables():
    """Make Exp and Ln resolve to the one table set containing both
    (natural_log_exp_and_others); otherwise the table-load pass alternates
    between exp_and_others and natural_log, reloading ~1.3us per activation.
    Set indices (= positions in act_info.json's act_func_sets) are preserved;
    only the membership used for set *selection* is filtered."""
    import concourse.bacc as bacc_mod
    from concourse import mybir

    if getattr(bacc_mod, "_act_tables_patched", False):
        return
    orig = bacc_mod.get_activation_tables
    exp_ln = {mybir.ActivationFunctionType.Exp, mybir.ActivationFunctionType.Ln}

    def patched(arch):
        out = {}
        for name, fns in orig(arch).items():
            out[name] = set(fns) if name == "natural_log_exp_and_others" else (
                set(fns) - exp_ln
            )
        return out

    bacc_mod.get_activation_tables = patched
    bacc_mod._act_tables_patched = True


def _ensure_ntff_hook():
    """Best-effort: make run_bass_kernel_spmd(trace=True) usable. The agent
    container ships no antenv.axon_hooks module, so a BASS_TRACE=1 run would
    otherwise die on the import inside bass_utils. Harmless if unused."""
    try:
        import types

        import antenv

        if "antenv.axon_hooks" in sys.modules:
            return
        m = types.ModuleType("antenv.axon_hooks")
        _h = {}
        m.set_axon_ntff_profile_hook = lambda h: _h.__setitem__("h", h)
        m.get_axon_ntff_profile_hook = lambda: _h.get("h")
        sys.modules["antenv.axon_hooks"] = m
        antenv.axon_hooks = m
        try:
            from trn_agent_boot.trn_boot import _ntff_profile_via_ctypes

            so = "/opt/axon/libaxon_pjrt.so"
            if os.path.exists(so):
                m.set_axon_ntff_profile_hook(_ntff_profile_via_ctypes(so))
        except Exception:
            pass
        try:
            import concourse.bass_utils as bu

            bu.upload_artifacts = lambda tmpdir: tmpdir
        except Exception:
            pass
    except Exception:
        pass


_CACHE = {}


def _get_nc():
    if "nc" not in _CACHE:
        import concourse.bacc as bacc

        _ensure_ntff_hook()
        _patch_act_tables()
        nc = bacc.Bacc("TRN2", target_bir_lowering=False, debug=False,
                       num_devices=N_CORES)
        build_program(nc)
        nc.compile()
        _CACHE["nc"] = nc
    return _CACHE["nc"]


def kernel(pred_boundary: np.ndarray, gt_mask: np.ndarray) -> np.ndarray:
    from concourse.bass_utils import run_bass_kernel_spmd

    nc = _get_nc()
    consts = make_consts()

    pred = np.ascontiguousarray(pred_boundary, dtype=np.float32).reshape(B * H, W)
    gt = np.ascontiguousarray(gt_mask, dtype=np.int32).reshape(B * H, W)

    rows_per_core = IMGS_PER_CORE * H
    in_maps = []
    for c in range(N_CORES):
        r0 = c * rows_per_core
        in_maps.append(
            {
                "pred": pred[r0 : r0 + rows_per_core],
                "gt": gt[r0 : r0 + rows_per_core],
                **consts,
            }
        )

    res = run_bass_kernel_spmd(nc, in_maps, list(range(N_CORES)))
    _CACHE["last_results"] = res

    fused_sp_cols = [2 * u for u in FUSED_UNITS]
    total = np.float64(0.0)
    for c in range(N_CORES):
        p = res.results[c]["partials"].astype(np.float64)
        sp = p[:, 0:SP_COLS].sum() - p[126, fused_sp_cols].sum()
        xz = p[:, SP_COLS : SP_COLS + N_UNITS].sum()
        total += sp - xz

    mean = total / float(B * C * H * W)
    return np.float32(mean)


# revision 9
# speedup vs baseline: 7.0477x; 1.0173x over previous
"""Boundary BCE loss kernel for Trainium2 (8 NeuronCores, data-parallel).

Computes mean(BCEWithLogits(pred, boundary(gt_mask))) where
boundary(m) = 1 iff the 3x3 neighborhood of a pixel (SAME window, valid
elements only) contains both a 0 and a 1.

Layout / algorithm (per core: 8 images of 1024x1024):
  - With *replicate* padding the value-set of a 3x3 window equals the set of
    valid in-bounds values. Weight the conv with center tap -8 (i.e.
    s' = replicate-pad 3x3 sum - 9*center): all-zeros -> s'=0, all-ones ->
    s'=9-9=0, mixed -> s' in [-8..-1] u [1..8]. So boundary z = (s' != 0),
    ONE vector op per tile, and sum(loss) = sum(softplus(x)) - sum(x*z).
  - The -9*center correction folds into the no-column-shift matmul's weight
    matrix (atop2/aint2/abst2), so the conv costs the same matmuls as the
    plain 3x3 count: 3 column-shifted matmuls per 512-col PSUM group (+ tiny
    N=1 edge-replicate matmuls reading mf's own edge columns).
  - Each image is row-tiled into 8 conv blocks of 128 input rows starting at
    126k (2-row overlap); block k=0 ("top") emits out rows 0..126 via a
    banded [128,127] matrix atop (replicate row -1 folded in), blocks
    k>=1 ("int") emit out rows 126k+1..126k+126 via aint whose column 126 is
    ZERO -- the resulting guaranteed s'=0 on partition 126 makes the x*z
    reduction contribute exactly 0 there, so reduction instructions can run
    on rectangular [127, 2048] tiles spanning a fused PAIR of blocks.
  - FUSION: blocks are processed in pairs (2pi, 2pi+1). One 3D-AP SWDGE
    *casting* DMA (int32->f8) loads both gt windows (row stride 126), so
    no engine ever spends time casting the mask; the two pred windows load
    as two plain 2D HWDGE DMAs (a 3D-AP HWDGE DMA costs ~4.2us of SP
    descriptor generation vs ~0.6us per 2D transfer).
  - exp/ln (softplus, Ln's free bias adds the +1) run fused [127,2048] for
    pairs pi>=1: partition 126 of each half then double-counts one row that
    the next block covers again; those sums land isolated in
    acc[126, fused-col] and the HOST subtracts that cell. Pair 0 runs
    exp/ln per-block (exact ranges), since its top half has no spare
    partition.
  - The 8 images' ragged bottom strips (16 in rows / 15 out rows each) are
    stacked into one [128, 1024] block via a 3D DMA and a block-diagonal
    matrix abst, exactly as a normal block.
  - TAIL: image 7's last two pairs are emitted as FOUR single-block units
    ([128,1024] tiles, exact exp/ln ranges), preceded by the bst unit. The
    last five units are all small, so the ACT engine keeps pace with the
    (end-of-stream bunched) x arrivals and the post-stream drain is one
    small unit (~3us) instead of ~7us.
  - Exp/Ln share one ACT table set (natural_log_exp_and_others; see
    _patch_act_tables) so tables load once. All six conv matrices arrive in
    one packed [128,768] DMA; all accumulators live in one [128,102] tile
    written back by a single output DMA.

Each core returns partials [128, 102]; the host sums in float64, subtracts
the fused-pair duplicate cells, and divides by N.
"""

import os
import sys
from collections import deque
from contextlib import ExitStack

import numpy as np

if "/opt/trn_rl_repo" not in sys.path and os.path.isdir("/opt/trn_rl_repo"):
    sys.path.append("/opt/trn_rl_repo")

N_CORES = 8
B, C, H, W = 64, 1, 1024, 1024
IMGS_PER_CORE = B // N_CORES  # 8
P = 128

N_PAIRS = 4          # fused block-pairs per image
MB = 15              # bottom strip out rows per image (1024 - (127+7*126))

# unit schedule: images 0-6 as fused pairs; image 7's last two pairs split
# into four single-block units ("blk": 0.5MB x + 0.5MB mf, ~2.3us ACT work)
# and interleaved with its remaining pairs and the bst unit so that, walking
# backward from the stream end, cumulative ACT work never exceeds cumulative
# DMA arrival time: the ACT engine carries no backlog into the stream tail
# and the post-stream drain is a single small unit (~3us) instead of ~8us.
UNITS = [("pair", img, pi) for img in range(IMGS_PER_CORE - 1)
         for pi in range(N_PAIRS)]
UNITS += [("bst",), ("pair", IMGS_PER_CORE - 1, 0), ("blk", 0),
          ("pair", IMGS_PER_CORE - 1, 1), ("blk", 1), ("blk", 2), ("blk", 3)]
N_UNITS = len(UNITS)                    # 35
SP_COLS = 2 * N_UNITS                   # softplus accum columns (2 per unit)
# units whose exp/ln run fused => host subtracts acc[126, 2*u]
FUSED_UNITS = [u for u, spec in enumerate(UNITS)
               if spec[0] == "pair" and spec[2] != 0]


def make_consts():
    """Banded vertical-conv matrices A[k, m] = weight of input row k in out m.
    The *2 variants subtract 9 at the center tap (in-row of out-row m) and are
    used for the no-column-shift matmul, yielding s' = 3x3sum - 9*center."""
    import ml_dtypes

    f8 = ml_dtypes.float8_e4m3fn

    atop = np.zeros((128, 127), np.float32)
    for m in range(127):
        for k in (m - 1, m, m + 1):
            if 0 <= k < 128:
                atop[k, m] += 1.0
    atop[0, 0] += 1.0  # replicate row -1 -> row 0
    atop2 = atop.copy()
    for m in range(127):
        atop2[m, m] -= 9.0

    aint = np.zeros((128, 127), np.float32)  # col 126 stays ZERO (guard)
    for m in range(126):
        for k in (m, m + 1, m + 2):
            aint[k, m] += 1.0
    aint2 = aint.copy()
    for m in range(126):
        aint2[m + 1, m] -= 9.0

    abot = np.zeros((MB + 1, MB), np.float32)
    for m in range(MB):
        for k in (m, m + 1, m + 2):
            if k <= MB:
                abot[k, m] += 1.0
    abot[MB, MB - 1] += 1.0  # replicate row h -> row h-1
    abot2 = abot.copy()
    for m in range(MB):
        abot2[m + 1, m] -= 9.0

    kbs = IMGS_PER_CORE * (MB + 1)
    mbs = IMGS_PER_CORE * MB
    abst = np.zeros((kbs, mbs), np.float32)
    abst2 = np.zeros((kbs, mbs), np.float32)
    for j in range(IMGS_PER_CORE):
        abst[j * (MB + 1) : (j + 1) * (MB + 1), j * MB : (j + 1) * MB] = abot
        abst2[j * (MB + 1) : (j + 1) * (MB + 1), j * MB : (j + 1) * MB] = abot2

    # pack all six into one [128, 768] tensor (128-col aligned views)
    conv_all = np.zeros((128, 768), np.float32)
    conv_all[:, 0:127] = atop
    conv_all[:, 128:255] = atop2
    conv_all[:, 256:383] = aint
    conv_all[:, 384:511] = aint2
    conv_all[:kbs, 512 : 512 + mbs] = abst
    conv_all[:kbs, 640 : 640 + mbs] = abst2

    return {"conv_all": conv_all.astype(f8)}


def build_program(nc, n_imgs=IMGS_PER_CORE, h=H, w=W):
    """Emit the per-core Tile program onto `nc` (a Bacc)."""
    import concourse.tile as tile
    from concourse import mybir
    from concourse.ap import AP

    f32 = mybir.dt.float32
    i32 = mybir.dt.int32
    bf16 = mybir.dt.bfloat16
    # never-read reduction outputs store as fp8: halves their SBUF write
    # bytes (which contend with DMA writes); the f32 accumulators carry the
    # real results, so these values are dead
    f8 = mybir.dt.float8e4

    rows = n_imgs * h
    kbs = n_imgs * (MB + 1)   # 128 stacked bottom-strip input rows
    mbs = n_imgs * MB         # 120 stacked bottom-strip output rows

    pred_d = nc.dram_tensor("pred", [rows, w], f32, kind="ExternalInput")
    gt_d = nc.dram_tensor("gt", [rows, w], i32, kind="ExternalInput")
    call_d = nc.dram_tensor("conv_all", [128, 768], f8, kind="ExternalInput")
    # partials: [0:SP_COLS) softplus sums, then N_UNITS x*z sums
    out_d = nc.dram_tensor("partials", [P, SP_COLS + N_UNITS], f32,
                           kind="ExternalOutput")

    pred = pred_d.ap()
    gt = gt_d.ap()
    pred3 = pred.rearrange("(j r) c -> j r c", j=n_imgs)
    gt3 = gt.rearrange("(j r) c -> j r c", j=n_imgs)
    out = out_d.ap()

    Exp = mybir.ActivationFunctionType.Exp
    Ln = mybir.ActivationFunctionType.Ln
    NE = mybir.AluOpType.not_equal
    MUL = mybir.AluOpType.mult

    with tile.TileContext(nc) as tc, ExitStack() as ctx:
        consts = ctx.enter_context(tc.tile_pool(name="consts", bufs=1))
        xs = ctx.enter_context(tc.tile_pool(name="xs", bufs=12))
        mfs = ctx.enter_context(tc.tile_pool(name="mfs", bufs=6))
        exs = ctx.enter_context(tc.tile_pool(name="exs", bufs=2))
        sps = ctx.enter_context(tc.tile_pool(name="sps", bufs=2))
        ws = ctx.enter_context(tc.tile_pool(name="ws", bufs=4))
        accp = ctx.enter_context(tc.tile_pool(name="accs", bufs=1))
        psum = ctx.enter_context(tc.tile_pool(name="psum", bufs=2, space="PSUM"))

        # one packed DMA on the scalar HWDGE ring: descriptor-gen must not
        # delay unit 0's x loads on the sync ring (ACT is idle until the
        # first x tile lands anyway)
        ct = consts.tile([128, 768], f8, tag="conv_all")
        nc.scalar.dma_start(ct[:], call_d.ap()[:])
        atop, atop2 = ct[:, 0:127], ct[:, 128:255]
        aint, aint2 = ct[:, 256:383], ct[:, 384:511]
        abst, abst2 = ct[:, 512 : 512 + mbs], ct[:, 640 : 640 + mbs]

        # single accumulator tile: softplus cols then x*z cols -> 1 out DMA
        acc = accp.tile([P, SP_COLS + N_UNITS], f32, tag="acc")
        nc.vector.memset(acc[:], 0.0)

        def conv_half(s2, ac, ao, mf2, Bo):
            """3x3 weighted conv of one 1024-col half: per 512-col PSUM group
            a center matmul with ac (= a - 9*centerband) + 2 column-shifted
            matmuls with ao + N=1 edge-replicate matmuls from mf's own edges.
            (Matmuls wider than 512 or not bank-aligned fail the ISA check
            s3d3_mm_num_elements, so per-512-col emission is mandatory.)"""
            mm = nc.tensor.matmul
            mm(s2[:, Bo + 0 : Bo + 512], ac[:], mf2[:, Bo + 0 : Bo + 512],
               start=True, stop=False)
            mm(s2[:, Bo + 0 : Bo + 512], ao[:], mf2[:, Bo + 1 : Bo + 513],
               start=False, stop=False)
            mm(s2[:, Bo + 1 : Bo + 512], ao[:], mf2[:, Bo + 0 : Bo + 511],
               start=False, stop=False)
            mm(s2[:, Bo + 0 : Bo + 1], ao[:], mf2[:, Bo + 0 : Bo + 1],
               start=False, stop=True)
            mm(s2[:, Bo + 512 : Bo + 1024], ac[:], mf2[:, Bo + 512 : Bo + 1024],
               start=True, stop=False)
            mm(s2[:, Bo + 512 : Bo + 1024], ao[:], mf2[:, Bo + 511 : Bo + 1023],
               start=False, stop=False)
            mm(s2[:, Bo + 512 : Bo + 1023], ao[:], mf2[:, Bo + 513 : Bo + 1024],
               start=False, stop=False)
            mm(s2[:, Bo + 1023 : Bo + 1024], ao[:], mf2[:, Bo + 1023 : Bo + 1024],
               start=False, stop=True)

        def front_pair(img, pi):
            """Loads + conv for fused block pair (2pi, 2pi+1) of one image."""
            in_r0 = img * h + 252 * pi
            mf2 = mfs.tile([128, 2048], f8, tag="mf")
            nc.gpsimd.dma_start(
                mf2[:],
                AP(gt.tensor, in_r0 * w, [(w, 128), (126 * w, 2), (1, w)]),
            )
            or0 = 0 if pi == 0 else 126 * 2 * pi + 1
            dlt = 127 if pi == 0 else 126
            xr0 = img * h + or0
            # two plain 2D loads: a single 3D-AP HWDGE DMA costs ~4.2us of SP
            # descriptor generation (vs ~0.7us per 2D), throttling the x feed.
            # NOTE: loads must be full 128 rows -- a 127-row transfer drops
            # off the DMA_DIRECT2D fast path and costs ~20us of SP time each.
            x2 = xs.tile([128, 2048], f32, tag="x")
            nc.sync.dma_start(x2[:, 0:1024], pred[xr0 : xr0 + 128, :])
            nc.sync.dma_start(x2[:, 1024:2048],
                              pred[xr0 + dlt : xr0 + dlt + 128, :])
            s2 = psum.tile([127, 2048], f32, tag="s")
            conv_half(s2, atop2 if pi == 0 else aint2,
                      atop if pi == 0 else aint, mf2, 0)
            conv_half(s2, aint2, aint, mf2, 1024)
            return s2, x2

        def front_bst():
            """Loads + conv for the stacked bottom strips of all 8 images."""
            mfb = mfs.tile([kbs, w], f8, tag="mf")
            nc.gpsimd.dma_start(mfb[:], gt3[:, h - (MB + 1) : h, :])
            xb = xs.tile([mbs, w], f32, tag="x")
            nc.sync.dma_start(xb[:], pred3[:, h - MB : h, :])
            sb = psum.tile([mbs, w], f32, tag="s")
            conv_half(sb, abst2, abst, mfb, 0)
            return sb, xb

        def front_blk(k):
            """Loads + conv for single block 4+k of the last image (tail)."""
            img = n_imgs - 1
            in_r0 = img * h + 252 * (N_PAIRS - 2) + 126 * k
            mf2 = mfs.tile([128, 2048], f8, tag="mf")
            nc.gpsimd.dma_start(
                mf2[:, 0:1024], AP(gt.tensor, in_r0 * w, [(w, 128), (1, w)])
            )
            xr0 = in_r0 + 1
            x2 = xs.tile([128, 2048], f32, tag="x")
            nc.sync.dma_start(x2[:, 0:1024], pred[xr0 : xr0 + 128, :])
            s2 = psum.tile([127, 2048], f32, tag="s")
            conv_half(s2, aint2, aint, mf2, 0)
            return s2, x2

        def red_xz(u, s2, x2, np_, nc_):
            """acc[:, SP_COLS+u] += sum_cols x * (s' != 0) over [np_, nc_]."""
            w1 = ws.tile([127, 2048], f8, tag="w1")
            nc.vector.scalar_tensor_tensor(
                w1[0:np_, 0:nc_], s2[0:np_, 0:nc_], 0.0, x2[0:np_, 0:nc_],
                NE, MUL,
                accum_out=acc[0:np_, SP_COLS + u : SP_COLS + u + 1],
            )

        def back_pair(u, fused, s2, x2):
            """Reductions for one fused pair: softplus sums + x*z sums."""
            # exp output in bf16: halves ACT's SBUF write+read bytes (which
            # contend with DMA writes in slow-HBM phases); e^x rounding is
            # random-sign and vanishes in the 67M-pixel mean
            ex2 = exs.tile([127, 2048], bf16, tag="ex")
            sp2 = sps.tile([127, 2048], f8, tag="sp")
            if fused:
                # partition 126 double-counts one row per half; the host
                # subtracts acc[126, 2u] (it contains ONLY those rows)
                nc.scalar.activation(ex2[:], x2[0:127, :], Exp)
                nc.scalar.activation(sp2[:], ex2[:], Ln, bias=1.0,
                                     accum_out=acc[0:127, 2 * u : 2 * u + 1])
            else:
                nc.scalar.activation(ex2[0:127, 0:1024], x2[0:127, 0:1024], Exp)
                nc.scalar.activation(ex2[0:126, 1024:2048], x2[0:126, 1024:2048],
                                     Exp)
                nc.scalar.activation(sp2[0:127, 0:1024], ex2[0:127, 0:1024], Ln,
                                     bias=1.0,
                                     accum_out=acc[0:127, 2 * u : 2 * u + 1])
                nc.scalar.activation(sp2[0:126, 1024:2048], ex2[0:126, 1024:2048],
                                     Ln, bias=1.0,
                                     accum_out=acc[0:126, 2 * u + 1 : 2 * u + 2])
            red_xz(u, s2, x2, 127, 2048)

        def back_bst(u, sb, xb):
            ex = exs.tile([127, 2048], bf16, tag="ex")
            sp = sps.tile([127, 2048], f8, tag="sp")
            nc.scalar.activation(ex[0:mbs, 0:1024], xb[:], Exp)
            nc.scalar.activation(sp[0:mbs, 0:1024], ex[0:mbs, 0:1024], Ln,
                                 bias=1.0,
                                 accum_out=acc[0:mbs, 2 * u : 2 * u + 1])
            w1 = ws.tile([127, 2048], f8, tag="w1")
            nc.vector.scalar_tensor_tensor(
                w1[0:mbs, 0:1024], sb[:], 0.0, xb[:], NE, MUL,
                accum_out=acc[0:mbs, SP_COLS + u : SP_COLS + u + 1],
            )

        def back_blk(u, s2, x2):
            """Single tail block: exact exp/ln over its 126 out rows."""
            ex = exs.tile([127, 2048], bf16, tag="ex")
            sp = sps.tile([127, 2048], f8, tag="sp")
            nc.scalar.activation(ex[0:126, 0:1024], x2[0:126, 0:1024], Exp)
            nc.scalar.activation(sp[0:126, 0:1024], ex[0:126, 0:1024], Ln,
                                 bias=1.0,
                                 accum_out=acc[0:126, 2 * u : 2 * u + 1])
            red_xz(u, s2, x2, 127, 1024)

        pending = deque()
        for u, spec in enumerate(UNITS):
            if spec[0] == "pair":
                _, img, pi = spec
                pending.append(("pair", u, pi != 0, front_pair(img, pi)))
            elif spec[0] == "bst":
                pending.append(("bst", u, False, front_bst()))
            else:
                pending.append(("blk", u, False, front_blk(spec[1])))
            if len(pending) > 2:
                kind, pu, fused, pf = pending.popleft()
                if kind == "pair":
                    back_pair(pu, fused, *pf)
                elif kind == "bst":
                    back_bst(pu, *pf)
                else:
                    back_blk(pu, *pf)
        while pending:
            kind, pu, fused, pf = pending.popleft()
            if kind == "pair":
                back_pair(pu, fused, *pf)
            elif kind == "bst":
                back_bst(pu, *pf)
            else:
                back_blk(pu, *pf)

        nc.sync.dma_start(out[:], acc[:])


def _patch_act_tables():
    """Make Exp and Ln resolve to the one table set containing both
    (natural_log_exp_and_others); otherwise the table-load pass alternates
    between exp_and_others and natural_log, reloading ~1.3us per activation.
    Set indices (= positions in act_info.json's act_func_sets) are preserved;
    only the membership used for set *selection* is filtered."""
    import concourse.bacc as bacc_mod
    from concourse import mybir

    if getattr(bacc_mod, "_act_tables_patched", False):
        return
    orig = bacc_mod.get_activation_tables
    exp_ln = {mybir.ActivationFunctionType.Exp, mybir.ActivationFunctionType.Ln}

    def patched(arch):
        out = {}
        for name, fns in orig(arch).items():
            out[name] = set(fns) if name == "natural_log_exp_and_others" else (
                set(fns) - exp_ln
            )
        return out

    bacc_mod.get_activation_tables = patched
    bacc_mod._act_tables_patched = True


def _ensure_ntff_hook():
    """Best-effort: make run_bass_kernel_spmd(trace=True) usable. The agent
    container ships no antenv.axon_hooks module, so a BASS_TRACE=1 run would
    otherwise die on the import inside bass_utils. Harmless if unused."""
    try:
        import types

        import antenv

        if "antenv.axon_hooks" in sys.modules:
            return
        m = types.ModuleType("antenv.axon_hooks")
        _h = {}
        m.set_axon_ntff_profile_hook = lambda h: _h.__setitem__("h", h)
        m.get_axon_ntff_profile_hook = lambda: _h.get("h")
        sys.modules["antenv.axon_hooks"] = m
        antenv.axon_hooks = m
        try:
            from trn_agent_boot.trn_boot import _ntff_profile_via_ctypes

            so = "/opt/axon/libaxon_pjrt.so"
            if os.path.exists(so):
                m.set_axon_ntff_profile_hook(_ntff_profile_via_ctypes(so))
        except Exception:
            pass
        try:
            import concourse.bass_utils as bu

            bu.upload_artifacts = lambda tmpdir: tmpdir
        except Exception:
            pass
    except Exception:
        pass


_CACHE = {}


def _get_nc():
    if "nc" not in _CACHE:
        import concourse.bacc as bacc

        _ensure_ntff_hook()
        _patch_act_tables()
        nc = bacc.Bacc("TRN2", target_bir_lowering=False, debug=False,
                       num_devices=N_CORES)
        build_program(nc)
        nc.compile()
        _CACHE["nc"] = nc
    return _CACHE["nc"]


def kernel(pred_boundary: np.ndarray, gt_mask: np.ndarray) -> np.ndarray:
    from concourse.bass_utils import run_bass_kernel_spmd

    nc = _get_nc()
    consts = make_consts()

    pred = np.ascontiguousarray(pred_boundary, dtype=np.float32).reshape(B * H, W)
    gt = np.ascontiguousarray(gt_mask, dtype=np.int32).reshape(B * H, W)

    rows_per_core = IMGS_PER_CORE * H
    in_maps = []
    for c in range(N_CORES):
        r0 = c * rows_per_core
        in_maps.append(
            {
                "pred": pred[r0 : r0 + rows_per_core],
                "gt": gt[r0 : r0 + rows_per_core],
                **consts,
            }
        )

    res = run_bass_kernel_spmd(nc, in_maps, list(range(N_CORES)))
    _CACHE["last_results"] = res

    fused_sp_cols = [2 * u for u in FUSED_UNITS]
    total = np.float64(0.0)
    for c in range(N_CORES):
        p = res.results[c]["partials"].astype(np.float64)
        sp = p[:, 0:SP_COLS].sum() - p[126, fused_sp_cols].sum()
        xz = p[:, SP_COLS : SP_COLS + N_UNITS].sum()
        total += sp - xz

    mean = total / float(B * C * H * W)
    return np.float32(mean)
